# revision 1
# baseline (speedup 1.0000x reference)
"""Additive (Bahdanau) attention on 8 TRN2 NeuronCores.

Problem shapes: B=4, Q=512, K=1024, Dq=Dk=Dv=512, H=128.
Sharding: data-parallel over batch x query-halves -> core c handles
batch c//2, query rows [(c%2)*256, (c%2)*256+256). Each core gets the
full keys/values of its batch (softmax reduces over all K), so no
cross-core collectives are needed.

Two implementations (env KERNEL_IMPL = "sine" (default) | "tanh"):

tanh: the direct algorithm. Per q, one scalar-engine activation computes
tanh(kfT + qfT[:, q]) over [H=128, K] (the q feature enters through the
activation's per-partition bias); the H-reduction against w_v is a
stationary-F matmul producing transposed score columns. The 268M-element
tanh makes the scalar engine the bottleneck (~33.5M elems/core,
~300 us/core modeled).

sine (~70 us/core modeled, ~4x faster): tanh(a+b) is separable through
the angle-addition identity. Fit tanh(x) ~ sum_r c_r sin(w_r x)
(weighted least squares, R=12, max err ~1e-3 over the feature-sum
range [-8.5, 8.5]), then

  scores[q,k] = sum_h w_h tanh(qf_hq + kf_hk)
             = sum_r [ (c_r w_h sin(w_r qf)) . cos(w_r kf)
                     + (c_r w_h cos(w_r qf)) . sin(w_r kf) ]   (contract h)

i.e. 2R=24 accumulating 128-contraction matmuls on the tensor engine
instead of 268M scalar-engine tanh evaluations. The ACT Sin table is
only accurate on [-pi, pi], so arguments are range-reduced exactly:

  t   = x * (w/2pi)                 (gpsimd tensor_scalar)
  a_s = fl(t + 1.5*2^23)            (gpsimd; the fp32 store rounds t to
                                     the nearest integer -- 2^23 alone
                                     fails for negative t, and a chained
                                     two-op tensor_scalar keeps extended
                                     precision so the round must be its
                                     own instruction)
  e_s = (a_s - 1.5*2^23) - t        (DVE scalar_tensor_tensor; both
                                     terms exact) = round(t) - t
  sin(w x) = sin(-2pi * e_s)        (ACT Sin, scale = -2pi)

and for cos, reusing t: a_c = fl((t + 1/4) + 1.5*2^23),
e_c = (a_c - 1.5*2^23) - t, cos(w x) = sin(-2pi*e_c + pi/2) with the
pi/2 supplied through the activation bias. The wrap chains run on the
otherwise-idle GPSIMD + vector engines, k-side split in 512-column
halves for pipelining; sin/cos tiles are bf16 (the c_r*w_h weighting is
folded into the q-side, keeping per-term magnitudes small, which makes
the bf16 quantization error SMALLER than in the direct tanh path).

Score accumulation note: matmul start=True clears the has_written bits
of the whole PSUM bank, so interleaved accumulation groups sharing a
bank cannot use it. A dummy full-bank start=True matmul writes zeros
and sets every bit; the 24 real matmuls per score region then
accumulate with start=False in r-streaming order (each r's matmuls run
as soon as its trig tiles are ready).

Softmax needs no max-subtraction (scores are O(1) by construction: w_v
has variance 1/H). exp runs once over the transposed scores [k, q] in
PSUM so the attn tile is directly the stationary operand of the attn@V
matmuls; the softmax denominator comes from one extra accumulating
matmul against a ones vector, followed by a DVE reciprocal and a
per-partition rescale of the output. DMA traffic is spread across both
hardware DGE queues (SP + Activation issuers), with the values load
priority-deferred off the keys->features critical path.
"""


import os
import numpy as np

import concourse.bass as bass
import concourse.mybir as mybir
import concourse.tile as tile
from concourse import bacc
from concourse.bass_utils import run_bass_kernel_spmd
from concourse.masks import make_identity

B, Q, K, D, H = 4, 512, 1024, 512, 128
N_CORES = 8
QSH = Q * B // N_CORES          # 256 query rows per core
NDC = D // 128                  # 4 contraction chunks
NKC = K // 128                  # 8 key chunks
NQB = QSH // 128                # 2 query blocks per core

F32 = mybir.dt.float32
BF16 = mybir.dt.bfloat16
TANH = mybir.ActivationFunctionType.Tanh
EXP = mybir.ActivationFunctionType.Exp
SIN = mybir.ActivationFunctionType.Sin
TS = mybir.AluOpType

MAGIC = 12582912.0              # 1.5 * 2**23: fp32 add forces round-to-int
TWO_PI = float(2.0 * np.pi)

LAST_EXEC_NS = None
_NC_CACHE = {}


R_SINE = 12
WMAX = 5.0

def _fit_sine(R=R_SINE, wmax=WMAX, L=8.5, sigma=1.7):
    """Least-squares fit tanh(x) ~ sum_r c_r sin(w_r x) on [-L, L]."""
    ws = np.linspace(wmax / R * 0.5, wmax, R)
    xs = np.linspace(-L, L, 4001)
    wt = np.exp(-xs ** 2 / (2 * sigma ** 2)) + 1e-3
    A = np.sin(np.outer(xs, ws))
    Wt = np.sqrt(wt)[:, None]
    c, *_ = np.linalg.lstsq(A * Wt, np.tanh(xs) * Wt[:, 0], rcond=None)
    return [float(w) for w in ws], [float(v) for v in c]


def _declare_io(nc):
    q_ext = nc.declare_dram_parameter("q", [QSH, D], F32, isOutput=False)
    k_ext = nc.declare_dram_parameter("k", [K, D], F32, isOutput=False)
    v_ext = nc.declare_dram_parameter("v", [K, D], F32, isOutput=False)
    wq_ext = nc.declare_dram_parameter("wq", [D, H], F32, isOutput=False)
    wk_ext = nc.declare_dram_parameter("wk", [D, H], F32, isOutput=False)
    wv_ext = nc.declare_dram_parameter("wv", [H, 1], F32, isOutput=False)
    out_ext = nc.declare_dram_parameter("out", [QSH, D], F32, isOutput=True)
    return q_ext, k_ext, v_ext, wq_ext, wk_ext, wv_ext, out_ext


def _preamble(nc, tc, const, work, feat, q_ext, k_ext, v_ext, wq_ext, wk_ext,
              wv_ext):
    """DMA + PE-transpose inputs, feature matmuls, values cast.

    Returns (qf_sb [H, QSH] f32, kf_sb [H, K] f32, v_b [128, NKC, D] bf16,
    wv_f [H,1] f32, ones_b [128,1] bf16)."""
    ident = const.tile([128, 128], F32)
    make_identity(nc, ident)

    wq_t = const.tile([128, NDC, H], F32)
    wk_t = const.tile([128, NDC, H], F32)
    nc.sync.dma_start(out=wq_t, in_=wq_ext.rearrange("(c p) h -> p c h", p=128))
    nc.sync.dma_start(out=wk_t, in_=wk_ext.rearrange("(c p) h -> p c h", p=128))

    wv_f = const.tile([H, 1], F32)
    nc.sync.dma_start(out=wv_f, in_=wv_ext[:])
    ones_b = const.tile([128, 1], BF16)
    nc.vector.memset(ones_b, 1.0)

    qT = feat.tile([128, NDC, QSH], F32)
    kT = feat.tile([128, NDC, K], F32)
    qf_sb = feat.tile([H, QSH], F32)
    kf_sb = feat.tile([H, K], F32)
    with tc.tile_pool(name="kwork", bufs=8) as kwork, \
         tc.tile_pool(name="pre_ps", bufs=2, space="PSUM") as pre_ps:
        for t in range(QSH // 128):
            q_in = work.tile([128, D], F32, tag="qin")
            nc.sync.dma_start(out=q_in, in_=q_ext[t * 128:(t + 1) * 128, :])
            for dc in range(NDC):
                tp = pre_ps.tile([128, 128], F32, tag="tps")
                nc.tensor.transpose(tp, q_in[:, dc * 128:(dc + 1) * 128], ident)
                dst = qT[:, dc, t * 128:(t + 1) * 128]
                (nc.vector.tensor_copy(dst, tp) if dc % 2 == 0
                 else nc.scalar.copy(dst, tp))

        for t in range(K // 128):
            k_in = kwork.tile([128, D], F32, tag="kin")
            dma_eng = (nc.sync, nc.scalar)[t % 2]
            dma_eng.dma_start(out=k_in, in_=k_ext[t * 128:(t + 1) * 128, :])
            for dc in range(NDC):
                tp = pre_ps.tile([128, 128], F32, tag="tps")
                nc.tensor.transpose(tp, k_in[:, dc * 128:(dc + 1) * 128], ident)
                dst = kT[:, dc, t * 128:(t + 1) * 128]
                (nc.vector.tensor_copy(dst, tp) if dc % 2 == 0
                 else nc.scalar.copy(dst, tp))

        qf_ps = pre_ps.tile([H, QSH], F32, tag="fps")
        for dc in range(NDC):
            nc.tensor.matmul(qf_ps, wq_t[:, dc, :], qT[:, dc, :],
                             start=(dc == 0), stop=(dc == NDC - 1))
        nc.vector.tensor_copy(qf_sb, qf_ps)

        for hf in range(2):
            kf_ps = pre_ps.tile([H, 512], F32, tag="fps")
            for dc in range(NDC):
                nc.tensor.matmul(kf_ps, wk_t[:, dc, :],
                                 kT[:, dc, hf * 512:(hf + 1) * 512],
                                 start=(dc == 0), stop=(dc == NDC - 1))
            dst = kf_sb[:, hf * 512:(hf + 1) * 512]
            (nc.vector.tensor_copy(dst, kf_ps) if hf == 0
             else nc.scalar.copy(dst, kf_ps))

    v_b = feat.tile([128, NKC, D], BF16)
    tc.tile_set_cur_wait(0.05)   # keep values off the keys->kf critical path
    for kc in range(NKC):
        v_in = work.tile([128, D], F32, tag="vin")
        (nc.sync, nc.scalar)[kc % 2].dma_start(
            out=v_in, in_=v_ext[kc * 128:(kc + 1) * 128, :])
        nc.gpsimd.tensor_copy(v_b[:, kc, :], v_in)
    tc.tile_set_cur_wait(0)

    return qf_sb, kf_sb, v_b, wv_f, ones_b


def _softmax_av_tail(nc, ps, oloop, sc_ps_qb, attnT, v_b, ones_b, out_ext, qb):
    """exp(scoresT) -> attnT bf16; attn@V + denom matmuls; scale; DMA out."""
    nc.scalar.activation(out=attnT, in_=sc_ps_qb, func=EXP)
    o_ps = ps.tile([128, D], F32, tag="ops")
    d_ps = ps.tile([128, 1], F32, tag="dps")
    for kc in range(NKC):
        nc.tensor.matmul(o_ps, attnT[:, kc, :], v_b[:, kc, :],
                         start=(kc == 0), stop=(kc == NKC - 1))
        nc.tensor.matmul(d_ps, attnT[:, kc, :], ones_b,
                         start=(kc == 0), stop=(kc == NKC - 1))
    recip = oloop.tile([128, 1], F32, tag="recip")
    nc.vector.reciprocal(recip, d_ps)
    o_sb = oloop.tile([128, D], F32, tag="osb")
    nc.vector.tensor_scalar_mul(o_sb, o_ps, recip)
    nc.sync.dma_start(out=out_ext[qb * 128:(qb + 1) * 128, :], in_=o_sb)


def _build_tanh():
    nc = bacc.Bacc()
    q_ext, k_ext, v_ext, wq_ext, wk_ext, wv_ext, out_ext = _declare_io(nc)

    with tile.TileContext(nc) as tc:
        with tc.tile_pool(name="const", bufs=1) as const, \
             tc.tile_pool(name="work", bufs=3) as work, \
             tc.tile_pool(name="feat", bufs=1) as feat, \
             tc.tile_pool(name="floop", bufs=4) as floop, \
             tc.tile_pool(name="trig", bufs=4) as trig, \
             tc.tile_pool(name="oloop", bufs=2) as oloop:

            qf_sb, kf_sb, v_b, wv_f, ones_b = _preamble(
                nc, tc, const, work, feat, q_ext, k_ext, v_ext,
                wq_ext, wk_ext, wv_ext)
            wv_b = const.tile([H, 1], BF16)
            nc.vector.tensor_copy(wv_b, wv_f)

            with tc.tile_pool(name="ps", bufs=2, space="PSUM") as ps:
                for qb in range(NQB):
                    scT = ps.tile([128, NKC, 128], F32, tag="scT")
                    for qq in range(128):
                        qi = qb * 128 + qq
                        f_t = floop.tile([H, K], BF16, tag="F")
                        nc.scalar.activation(out=f_t, in_=kf_sb, func=TANH,
                                             bias=qf_sb[:, qi:qi + 1], scale=1.0)
                        for kc in range(NKC):
                            nc.tensor.matmul(scT[:, kc, qq:qq + 1],
                                             f_t[:, kc * 128:(kc + 1) * 128],
                                             wv_b, start=True, stop=True)
                    attnT = oloop.tile([128, NKC, 128], BF16, tag="attnT")
                    _softmax_av_tail(nc, ps, oloop, scT, attnT, v_b, ones_b,
                                     out_ext, qb)
    nc.compile()
    return nc


def _build_sine():
    debug = bool(os.environ.get("KERNEL_DEBUG"))
    ws, cs = _fit_sine()
    R = len(ws)
    nc = bacc.Bacc()
    q_ext, k_ext, v_ext, wq_ext, wk_ext, wv_ext, out_ext = _declare_io(nc)
    if debug:
        dbg = {
            "dbg_kf": nc.declare_dram_parameter("dbg_kf", [H, K], F32, isOutput=True),
            "dbg_qf": nc.declare_dram_parameter("dbg_qf", [H, QSH], F32, isOutput=True),
            "dbg_ks": nc.declare_dram_parameter("dbg_ks", [H, K], BF16, isOutput=True),
            "dbg_kc": nc.declare_dram_parameter("dbg_kc", [H, K], BF16, isOutput=True),
            "dbg_qs": nc.declare_dram_parameter("dbg_qs", [H, QSH], BF16, isOutput=True),
            "dbg_qc": nc.declare_dram_parameter("dbg_qc", [H, QSH], BF16, isOutput=True),
            "dbg_sc": nc.declare_dram_parameter("dbg_sc", [128, NQB, NKC, 128], F32, isOutput=True),
        }

    with tile.TileContext(nc) as tc:
        with tc.tile_pool(name="const", bufs=1) as const, \
             tc.tile_pool(name="work", bufs=3) as work, \
             tc.tile_pool(name="feat", bufs=1) as feat, \
             tc.tile_pool(name="trig", bufs=4) as trig, \
             tc.tile_pool(name="oloop", bufs=2) as oloop:

            qf_sb, kf_sb, v_b, wv_f, ones_b = _preamble(
                nc, tc, const, work, feat, q_ext, k_ext, v_ext,
                wq_ext, wk_ext, wv_ext)

            # per-r q-side coefficient vectors: wc[:, r] = c_r * w_v
            wc = const.tile([H, R], F32)
            for r in range(R):
                nc.vector.tensor_scalar_mul(wc[:, r:r + 1], wv_f, float(cs[r]))

            KS = feat.tile([H, R, K], BF16)
            KC = feat.tile([H, R, K], BF16)
            QS = feat.tile([H, R, QSH], BF16)
            QC = feat.tile([H, R, QSH], BF16)

            halfpi = const.tile([H, 1], F32)
            nc.vector.memset(halfpi, float(np.pi / 2))

            def trig_chain(x_sl, width, out_s, out_c, scale_col):
                """out_s = sin(w*x), out_c = cos(w*x), via exact range
                reduction: t = x*(w/2pi); a = fl(t + 1.5*2^23) (rounds to
                int at the fp32 store); e = (a - MAGIC) - t = round(t) - t
                (both terms exact); sin(w*x) = sin(-2pi*e). For cos, reuse
                t: a_c = fl((t + 1/4) + MAGIC), e_c = (a_c - MAGIC) - t =
                round(t + 1/4) - t, and cos(w*x) = sin(2pi*(t + 1/4)) =
                sin(-2pi*e_c + pi/2) -- the pi/2 lands in the ACT bias."""
                pool = trig
                t_t = pool.tile([H, width], F32, tag=f"t{width}")
                nc.gpsimd.tensor_scalar(t_t, x_sl, w2p, None, TS.mult)
                a_s = pool.tile([H, width], F32, tag=f"as{width}")
                nc.gpsimd.tensor_scalar(a_s, t_t, MAGIC, None, TS.add)
                a_c = pool.tile([H, width], F32, tag=f"ac{width}")
                nc.gpsimd.tensor_scalar(a_c, t_t, 0.25, MAGIC, TS.add, TS.add)
                e_s = pool.tile([H, width], F32, tag=f"es{width}")
                nc.vector.scalar_tensor_tensor(e_s, a_s, MAGIC, t_t,
                                               TS.subtract, TS.subtract)
                e_c = pool.tile([H, width], F32, tag=f"ec{width}")
                nc.vector.scalar_tensor_tensor(e_c, a_c, MAGIC, t_t,
                                               TS.subtract, TS.subtract)
                if scale_col is None:
                    nc.scalar.activation(out=out_s, in_=e_s, func=SIN,
                                         scale=-TWO_PI)
                    nc.scalar.activation(out=out_c, in_=e_c, func=SIN,
                                         scale=-TWO_PI, bias=halfpi[:, 0:1])
                else:
                    s_t = trig.tile([H, width], F32, tag=f"ss{width}")
                    nc.scalar.activation(out=s_t, in_=e_s, func=SIN,
                                         scale=-TWO_PI)
                    nc.gpsimd.tensor_scalar_mul(out_s, s_t, scale_col)
                    c_t = trig.tile([H, width], F32, tag=f"sc{width}")
                    nc.scalar.activation(out=c_t, in_=e_c, func=SIN,
                                         scale=-TWO_PI, bias=halfpi[:, 0:1])
                    nc.gpsimd.tensor_scalar_mul(out_c, c_t, scale_col)

            for r in range(R):
                w2p = float(ws[r] / TWO_PI)
                wcol = wc[:, r:r + 1]
                for hk in range(2):
                    sl = slice(hk * 512, (hk + 1) * 512)
                    trig_chain(kf_sb[:, sl], 512, KS[:, r, sl], KC[:, r, sl],
                               None)
                trig_chain(qf_sb, QSH, QS[:, r, :], QC[:, r, :], wcol)

            if debug:
                nc.sync.dma_start(out=dbg["dbg_kf"][:], in_=kf_sb)
                nc.sync.dma_start(out=dbg["dbg_qf"][:], in_=qf_sb)
                nc.sync.dma_start(out=dbg["dbg_ks"][:], in_=KS[:, 0, :])
                nc.sync.dma_start(out=dbg["dbg_kc"][:], in_=KC[:, 0, :])
                nc.sync.dma_start(out=dbg["dbg_qs"][:], in_=QS[:, 0, :])
                nc.sync.dma_start(out=dbg["dbg_qc"][:], in_=QC[:, 0, :])

            zeros_b = const.tile([128, 512], BF16)
            nc.vector.memset(zeros_b, 0.0)

            with tc.tile_pool(name="ps", bufs=2, space="PSUM") as ps, \
                 tc.tile_pool(name="ps4", bufs=1, space="PSUM") as ps4:
                sc_ps = ps4.tile([128, NKC, QSH], F32)
                # start=True clears has_written for the WHOLE bank, so
                # interleaved accumulation groups sharing a bank must not use
                # it. Instead: one full-bank dummy start=True matmul writes
                # zeros + sets every has_written bit; all real matmuls then
                # accumulate with start=False in r-streaming order.
                for bank in range(NKC * QSH // 512):
                    region = sc_ps[:, 2 * bank:2 * bank + 2, :]
                    nc.tensor.matmul(region, zeros_b[:, :128], zeros_b,
                                     start=True, stop=False,
                                     skip_group_check=True)
                for r in range(R):
                    for kc in range(NKC):
                        ksl = slice(kc * 128, (kc + 1) * 128)
                        nc.tensor.matmul(sc_ps[:, kc, :],
                                         KC[:, r, ksl], QS[:, r, :],
                                         start=False, stop=False,
                                         skip_group_check=True)
                        nc.tensor.matmul(sc_ps[:, kc, :],
                                         KS[:, r, ksl], QC[:, r, :],
                                         start=False, stop=(r == R - 1),
                                         skip_group_check=True)
                if debug:
                    for qb in range(NQB):
                        sct = work.tile([128, NKC, 128], F32, tag="dbgsc")
                        nc.vector.tensor_copy(sct, sc_ps[:, :, qb * 128:(qb + 1) * 128])
                        nc.sync.dma_start(out=dbg["dbg_sc"][:, qb], in_=sct)
                for qb in range(NQB):
                    qsl = slice(qb * 128, (qb + 1) * 128)
                    attnT = oloop.tile([128, NKC, 128], BF16, tag="attnT")
                    nc.scalar.activation(out=attnT, in_=sc_ps[:, :, qsl],
                                         func=EXP)
                    o_ps = ps.tile([128, D], F32, tag="ops")
                    d_ps = ps.tile([128, 1], F32, tag="dps")
                    for kc in range(NKC):
                        nc.tensor.matmul(o_ps, attnT[:, kc, :], v_b[:, kc, :],
                                         start=(kc == 0), stop=(kc == NKC - 1))
                        nc.tensor.matmul(d_ps, attnT[:, kc, :], ones_b,
                                         start=(kc == 0), stop=(kc == NKC - 1))
                    recip = oloop.tile([128, 1], F32, tag="recip")
                    nc.vector.reciprocal(recip, d_ps)
                    o_sb = oloop.tile([128, D], F32, tag="osb")
                    nc.vector.tensor_scalar_mul(o_sb, o_ps, recip)
                    (nc.sync, nc.scalar)[qb % 2].dma_start(
                        out=out_ext[qb * 128:(qb + 1) * 128, :], in_=o_sb)
    nc.compile()
    return nc


def _get_nc():
    impl = os.environ.get("KERNEL_IMPL", "sine")
    if impl not in _NC_CACHE:
        _NC_CACHE[impl] = _build_sine() if impl == "sine" else _build_tanh()
    return _NC_CACHE[impl]


def make_in_maps(queries, keys, values, W_q, W_k, w_v):
    queries = np.asarray(queries, dtype=np.float32)
    keys = np.asarray(keys, dtype=np.float32)
    values = np.asarray(values, dtype=np.float32)
    W_q = np.ascontiguousarray(np.asarray(W_q, dtype=np.float32))
    W_k = np.ascontiguousarray(np.asarray(W_k, dtype=np.float32))
    wv2 = np.ascontiguousarray(np.asarray(w_v, dtype=np.float32).reshape(H, 1))
    in_maps = []
    for c in range(N_CORES):
        b, qh = c // 2, c % 2
        in_maps.append({
            "q": np.ascontiguousarray(queries[b, qh * QSH:(qh + 1) * QSH, :]),
            "k": np.ascontiguousarray(keys[b]),
            "v": np.ascontiguousarray(values[b]),
            "wq": W_q,
            "wk": W_k,
            "wv": wv2,
        })
    return in_maps


def kernel(queries, keys, values, W_q, W_k, w_v):
    global LAST_EXEC_NS
    nc = _get_nc()
    in_maps = make_in_maps(queries, keys, values, W_q, W_k, w_v)

    trace = bool(os.environ.get("KERNEL_TRACE"))
    if trace:
        try:
            res = run_bass_kernel_spmd(nc, in_maps, core_ids=list(range(N_CORES)),
                                       trace=True)
            LAST_EXEC_NS = res.exec_time_ns
        except Exception:
            res = run_bass_kernel_spmd(nc, in_maps, core_ids=list(range(N_CORES)))
    else:
        res = run_bass_kernel_spmd(nc, in_maps, core_ids=list(range(N_CORES)))

    out = np.empty((B, Q, D), dtype=np.float32)
    for c in range(N_CORES):
        b, qh = c // 2, c % 2
        out[b, qh * QSH:(qh + 1) * QSH, :] = res.results[c]["out"]
    return out

